# revision 1
# baseline (speedup 1.0000x reference)
"""AdaAtt attention kernel for 8 Trainium2 NeuronCores (v2).

Pure data-parallel: batch B=2048 sharded 256 rows/core; weights replicated.
Per core (R=A=1024, G=49):

    fr  = relu(fake_region @ Wf1.T + bf1)
    fre = fr @ Wf2.T + bf2
    hl  = tanh(h_out @ Wh1.T + bh1)
    he  = hl @ Wh2.T + bh2
    scores[g] = Wa . tanh(embed[g] + he)     embed = [conv_feat_embed, fre]
    PI = softmax(scores)
    visAtt = sum_g PI[g] * img[g]            img = [fr, conv_feat]
    out = tanh((visAtt + hl) @ W2h.T + b2h)

v2 strategy: all dense chains run transposed ([feature-part, batch]) off
host-transposed xh/xf, making biases per-partition (free via ACT bias/scale)
and activations directly usable as matmul operands. conv_feat_embed is
host-transposed to [A, nb, G, 128b] fp8 with sign(Wa) folded in; the score
dot runs on the PE as fp8 DoubleRow matmuls with 8|Wa| stationary; score
rows [1, g*b] scatter back to [b, g] via a tiny element DMA. Softmax skips
the max-shift (scores bounded); visAtt uses unnormalized exp weights as fp8
diag DoubleRow matmuls over g-pairs of the fp8 conv stream, with the 1/sum
folded into the PSUM evacuation. Score columns: 0..48 = conv g, 49 = fre.
"""
import numpy as np
import ml_dtypes
from contextlib import ExitStack

import concourse.bass as bass
import concourse.mybir as mybir
import concourse.tile as tile
from concourse import bacc
from concourse.bass_utils import run_bass_kernel_spmd
from concourse.masks import make_identity

BF16 = mybir.dt.bfloat16
F32 = mybir.dt.float32
FP8 = mybir.dt.float8e4
AF = mybir.ActivationFunctionType
ALU = mybir.AluOpType
DR = mybir.MatmulPerfMode.DoubleRow

N_CORES = 8
B, R, A, G = 2048, 1024, 1024, 49
BS = B // N_CORES          # 256 rows per core
KC = R // 128              # 8 feature chunks
GB = 4                     # g's per block
NSC = G + 1                # score columns (49 conv + fre)

# phase-2 g-blocks over conv ids: 12x4 + [48, fre]
P2_BLOCKS = [list(range(s, s + GB)) for s in range(0, 48, GB)] + [[48, -2]]
# phase-3 (visAtt) slots: -1 = fr, else conv id. 25 g-pairs total.
P3_BLOCKS = [[-1, 0, 1, 2]] + [list(range(s, s + GB)) for s in range(3, 47, GB)] \
    + [[47, 48]]

_CACHED_NC = None


def _build():
    nc = bacc.Bacc("TRN2", target_bir_lowering=False, debug=False,
                   num_devices=N_CORES)

    xh_d = nc.dram_tensor("xh", [R, BS], BF16, kind="ExternalInput").ap()
    xf_d = nc.dram_tensor("xf", [R, BS], BF16, kind="ExternalInput").ap()
    # sign-folded, transposed embed: [A, nb, G, 128]
    cfes_d = nc.dram_tensor("cfes", [A, 2, G, 128], FP8,
                            kind="ExternalInput").ap()
    cf_d = nc.dram_tensor("cf", [BS, G, R], FP8, kind="ExternalInput").ap()
    w_d = {}
    for name in ("wh1t", "wh2t", "wf1t", "wf2t", "w2ht"):
        w_d[name] = nc.dram_tensor(name, [R, R], BF16, kind="ExternalInput").ap()
    # bias pack [128, 40]: bh1T, s*bh2T, bf1T, s*bf2T, sgn  (8 cols each)
    bp_d = nc.dram_tensor("bp", [128, 40], F32, kind="ExternalInput").ap()
    awa_d = nc.dram_tensor("awa", [128, KC, 128], FP8, kind="ExternalInput").ap()
    b2h_d = nc.dram_tensor("b2h", [1, R], BF16, kind="ExternalInput").ap()
    out_d = nc.dram_tensor("out", [BS, R], F32, kind="ExternalOutput").ap()
    nblk = len(P2_BLOCKS)
    sc_d = nc.dram_tensor("scsc", [2, nblk, GB, 128], F32, kind="Internal").ap()

    cfes_r = cfes_d.rearrange("(c p) nb g b -> p c nb g b", p=128)

    with ExitStack() as ctx:
        tc = ctx.enter_context(tile.TileContext(nc))
        consts = ctx.enter_context(tc.tile_pool(name="consts", bufs=1))
        wpool = ctx.enter_context(tc.tile_pool(name="wpool", bufs=4))
        acts = ctx.enter_context(tc.tile_pool(name="acts", bufs=1))
        cfep = ctx.enter_context(tc.tile_pool(name="cfep", bufs=2))
        hsp = ctx.enter_context(tc.tile_pool(name="hsp", bufs=2))
        thp = ctx.enter_context(tc.tile_pool(name="thp", bufs=3))
        cv0p = ctx.enter_context(tc.tile_pool(name="cv0p", bufs=2))
        cvp = ctx.enter_context(tc.tile_pool(name="cvp", bufs=10))
        dgp_pool = ctx.enter_context(tc.tile_pool(name="dgp", bufs=3))
        small = ctx.enter_context(tc.tile_pool(name="small", bufs=4))
        dense_ps = ctx.enter_context(tc.tile_pool(name="dense_ps", bufs=2,
                                                  space="PSUM"))
        wa_ps = ctx.enter_context(tc.tile_pool(name="wa_ps", bufs=2,
                                               space="PSUM"))
        tp_ps = ctx.enter_context(tc.tile_pool(name="tp_ps", bufs=2,
                                               space="PSUM"))
        vp_ps = ctx.enter_context(tc.tile_pool(name="vp_ps", bufs=1,
                                               space="PSUM"))

        # ---- constants / small inputs ----
        ident = consts.tile([128, 128], BF16)
        make_identity(nc, ident)
        ones1 = consts.tile([1, 128], BF16)
        nc.vector.memset(ones1, 1.0)
        bp = consts.tile([128, 40], F32)
        nc.sync.dma_start(out=bp, in_=bp_d)
        awa = consts.tile([128, KC, 128], FP8)
        nc.sync.dma_start(out=awa, in_=awa_d)
        b2h_sb = consts.tile([1, R], BF16)
        nc.sync.dma_start(out=b2h_sb, in_=b2h_d)

        # ---- activations (transposed: [128 feat-part, chunk, 256 batch]) ----
        xh_sb = acts.tile([128, KC, BS], BF16)
        nc.sync.dma_start(out=xh_sb,
                          in_=xh_d.rearrange("(c p) b -> p c b", p=128))
        hl_T = acts.tile([128, KC, BS], BF16)
        he_s = acts.tile([128, KC, BS], BF16)     # sign * (he + bh2)
        xf_sb = acts.tile([128, KC, BS], BF16)
        fr_T = acts.tile([128, KC, BS], BF16)
        fre_s = acts.tile([128, KC, BS], BF16)    # sign * (fre + bf2)
        sum_T = acts.tile([128, KC, BS], BF16)    # (visAtt + hl) transposed
        expw = acts.tile([128, 2, NSC], F32)
        rs = acts.tile([128, 2], F32)
        vps = acts.tile([128, R], BF16)           # scaled visAtt (reused per nb)
        out_sb = acts.tile([128, 2, R], F32)

        w_sb = {}

        def load_w(name):
            w_sb[name] = wpool.tile([128, KC, R], BF16, name=name, tag="w")
            nc.sync.dma_start(out=w_sb[name],
                              in_=w_d[name].rearrange("(c p) n -> p c n", p=128))

        load_w("wh1t")
        load_w("wh2t")

        def dense_T(wname, x_sb, o_sb, func, bias_col, scale_col=None):
            """o_sb[:, rc, :] = func(scale*( (W.T x).T )[rc] + bias), transposed.

            x_sb: [128, KC, BS] rhs chunks; weight slab w_sb[wname] [128, KC, R].
            bias_col/scale_col: fns rc -> [128, 1] column AP.
            """
            w = w_sb[wname]
            for rp in range(KC // 2):        # rc pairs share a psum tile
                ph = dense_ps.tile([128, 2, BS], F32, tag="dps")
                for i in range(2):
                    rc = rp * 2 + i
                    for kc in range(KC):
                        nc.tensor.matmul(
                            ph[:, i, :],
                            lhsT=w[:, kc, rc * 128:(rc + 1) * 128],
                            rhs=x_sb[:, kc, :],
                            start=(kc == 0), stop=(kc == KC - 1))
                for i in range(2):
                    rc = rp * 2 + i
                    sc = 1.0 if scale_col is None else scale_col(rc)
                    nc.scalar.activation(
                        out=o_sb[:, rc, :], in_=ph[:, i, :], func=func,
                        bias=bias_col(rc), scale=sc)

        # ---- h-chain (gates phase 2) ----
        dense_T("wh1t", xh_sb, hl_T, AF.Tanh, lambda rc: bp[:, rc:rc + 1])
        dense_T("wh2t", hl_T, he_s, AF.Identity,
                bias_col=lambda rc: bp[:, 8 + rc:8 + rc + 1],
                scale_col=lambda rc: bp[:, 32 + rc:32 + rc + 1])

        # conv stream: first block per nb lands beside fr in cv0
        cv0 = [cv0p.tile([128, GB, R], FP8, tag="cv0", name=f"cv0_{nb}")
               for nb in range(2)]
        for nb in range(2):
            nc.gpsimd.dma_start(
                out=cv0[nb][:, 1:4, :],
                in_=cf_d[nb * 128:(nb + 1) * 128, 0:3, :])

        # ---- phase 2: scores ----
        def p2_block(nb, bi, blk):
            """One scores block: conv ids (or -2 = fre slot) in blk."""
            bsl = slice(nb * 128, (nb + 1) * 128)
            gb = len(blk)
            ng = sum(1 for g in blk if g >= 0)       # conv slots (prefix)
            th = thp.tile([128, KC, GB, 128], FP8, tag="th", name="th")
            if ng:
                cfe = cfep.tile([128, KC, GB, 128], FP8, tag="cfe", name="cfe")
                nc.sync.dma_start(
                    out=cfe[:, :, :ng, :],
                    in_=cfes_r[:, :, nb, blk[0]:blk[0] + ng, :])
                hs = hsp.tile([128, KC, GB, 128], BF16, tag="hs", name="hs")
                for c in range(KC):
                    he_col = he_s[:, c, bsl].unsqueeze(1).to_broadcast(
                        [128, ng, 128])
                    nc.vector.tensor_add(hs[:, c, :ng, :], cfe[:, c, :ng, :],
                                         he_col)
                nc.scalar.activation(out=th[:, :, :ng, :],
                                     in_=hs[:, :, :ng, :], func=AF.Tanh)
            if gb > ng:  # fre slot (last)
                hsf = hsp.tile([128, KC, 128], BF16, tag="hsf", name="hsf",
                               bufs=2)
                for c in range(KC):
                    nc.vector.tensor_add(hsf[:, c, :], fre_s[:, c, bsl],
                                         he_s[:, c, bsl])
                nc.scalar.activation(out=th[:, :, ng, :], in_=hsf,
                                     func=AF.Tanh)
            # wa-dot: per g-pair, fp8 DoubleRow accumulated over c-pairs
            sp = wa_ps.tile([128, GB, 128], F32, tag="wa")
            for gp in range(gb // 2):
                for cp in range(KC // 2):
                    nc.tensor.matmul(
                        sp[:, gp * 2:gp * 2 + 2, :],
                        lhsT=awa[:, 2 * cp:2 * cp + 2, :],
                        rhs=th[:, 2 * cp:2 * cp + 2, gp * 2:gp * 2 + 2, :],
                        perf_mode=DR,
                        start=(cp == 0), stop=(cp == KC // 2 - 1))
            # exp-evac rows psum -> sbuf (ACT; scores are 8x true), then
            # bounce via DRAM to transpose [1, (g, b)] -> expw[b, nb, col]
            row = small.tile([1, GB, 128], F32, tag="slin", name="slin",
                             bufs=3)[:, :gb, :]
            nc.scalar.activation(out=row, in_=sp[0:1, :gb, :],
                                 func=AF.Exp, scale=0.125)
            nc.sync.dma_start(out=sc_d[nb:nb + 1, bi, :gb, :], in_=row)
            nc.sync.dma_start(
                out=expw[:, nb, blk[0]:blk[0] + gb],
                in_=sc_d[nb, bi, :gb, :].transpose([1, 0]))

        for bi, blk in enumerate(P2_BLOCKS[:-1]):
            for nb in range(2):
                p2_block(nb, bi, blk)
            if bi == 1:
                nc.sync.dma_start(
                    out=xf_sb, in_=xf_d.rearrange("(c p) b -> p c b", p=128))
                load_w("wf1t")
                load_w("wf2t")
            if bi == 5:
                dense_T("wf1t", xf_sb, fr_T, AF.Relu,
                        lambda rc: bp[:, 16 + rc:16 + rc + 1])
                dense_T("wf2t", fr_T, fre_s, AF.Identity,
                        bias_col=lambda rc: bp[:, 24 + rc:24 + rc + 1],
                        scale_col=lambda rc: bp[:, 32 + rc:32 + rc + 1])
                # fr -> natural fp8 into slot 0 of the first conv tile per nb
                for nb in range(2):
                    for rp in range(2):
                        tp = tp_ps.tile([128, 4, 128], BF16, tag="tp")
                        for i in range(4):
                            rc = rp * 4 + i
                            nc.tensor.transpose(
                                tp[:, i, :],
                                fr_T[:, rc, nb * 128:(nb + 1) * 128], ident)
                        nc.scalar.activation(
                            out=cv0[nb][:, 0, rp * 512:(rp + 1) * 512],
                            in_=tp, func=AF.Copy)
            if bi == 8:
                load_w("w2ht")
        for nb in range(2):
            p2_block(nb, len(P2_BLOCKS) - 1, P2_BLOCKS[-1])

        # ---- softmax tail (exp already applied on the score rows) ----
        sume = small.tile([128, 2], F32, tag="sume")
        for nb in range(2):
            nc.vector.tensor_reduce(sume[:, nb:nb + 1], expw[:, nb, :],
                                    axis=mybir.AxisListType.X, op=ALU.add)
        nc.vector.reciprocal(rs, sume)

        # ---- phase 3: visAtt = (sum_g exp_g img_g) * rs, then final dense ----
        for nb in range(2):
            bsl = slice(nb * 128, (nb + 1) * 128)
            vp = vp_ps.tile([128, R], F32, tag="vp", name=f"vp{nb}")
            pair_i = 0
            for ci, blk in enumerate(P3_BLOCKS):
                if ci == 0:
                    cv = cv0[nb]
                else:
                    cv = cvp.tile([128, GB, R], FP8, tag="cv", name="cv")
                    nc.gpsimd.dma_start(
                        out=cv[:, :len(blk), :],
                        in_=cf_d[bsl, blk[0]:blk[-1] + 1, :])
                for pj in range(len(blk) // 2):
                    dgp = dgp_pool.tile([128, 2, 128], FP8, tag="dg")
                    for j in range(2):
                        v = blk[pj * 2 + j]
                        col = NSC - 1 if v == -1 else v
                        nc.vector.tensor_scalar_mul(
                            dgp[:, j, :], ident, expw[:, nb, col:col + 1])
                    last = (ci == len(P3_BLOCKS) - 1 and
                            pj == len(blk) // 2 - 1)
                    for h in range(2):
                        nc.tensor.matmul(
                            vp[:, h * 512:(h + 1) * 512],
                            lhsT=dgp,
                            rhs=cv[:, pj * 2:pj * 2 + 2,
                                   h * 512:(h + 1) * 512],
                            perf_mode=DR,
                            start=(pair_i == 0), stop=last)
                    pair_i += 1

            # vps = rs * vp ; sum_T = transpose(vps) + hl_T ; final dense
            nc.scalar.activation(out=vps, in_=vp, func=AF.Copy,
                                 scale=rs[:, nb:nb + 1])
            for rp in range(2):
                tp = tp_ps.tile([128, 4, 128], BF16, tag="tp")
                for i in range(4):
                    rc = rp * 4 + i
                    nc.tensor.transpose(tp[:, i, :],
                                        vps[:, rc * 128:(rc + 1) * 128], ident)
                for i in range(4):
                    rc = rp * 4 + i
                    nc.vector.tensor_add(sum_T[:, rc, bsl], tp[:, i, :],
                                         hl_T[:, rc, bsl])
            for n in range(2):
                yp = dense_ps.tile([128, 2, BS], F32, tag="dps", name="ypf")
                for kc in range(KC):
                    nc.tensor.matmul(
                        yp, lhsT=sum_T[:, kc, bsl],
                        rhs=w_sb["w2ht"][:, kc, n * 512:(n + 1) * 512],
                        start=(kc == 0), stop=False)
                nc.tensor.matmul(yp, lhsT=ones1,
                                 rhs=b2h_sb[:, n * 512:(n + 1) * 512],
                                 start=False, stop=True)
                nc.scalar.activation(
                    out=out_sb[:, nb, n * 512:(n + 1) * 512],
                    in_=yp, func=AF.Tanh)
            nc.sync.dma_start(
                out=out_d.rearrange("(nb p) n -> p nb n", p=128)[:, nb, :],
                in_=out_sb[:, nb, :])

    nc.compile()
    return nc


def _get_nc():
    global _CACHED_NC
    if _CACHED_NC is None:
        _CACHED_NC = _build()
    return _CACHED_NC


def _prep_inputs(h_out, fake_region, conv_feat, conv_feat_embed,
                 Wf1, bf1, Wf2, bf2, Wh1, bh1, Wh2, bh2, Wa, ba, W2h, b2h):
    bf = ml_dtypes.bfloat16
    f8 = ml_dtypes.float8_e4m3
    f32 = np.float32

    wa = np.asarray(Wa, f32).reshape(A)
    sgn = np.where(wa >= 0, 1.0, -1.0).astype(f32)
    awa8 = (8.0 * np.abs(wa)).reshape(KC, 128).T  # [128, KC]
    # replicate across 128 stationary columns (dual-fp8 LS wants full form)
    awa8 = np.ascontiguousarray(
        np.broadcast_to(awa8[:, :, None], (128, KC, 128)))
    sgnT = sgn.reshape(KC, 128).T

    def bT(b, s=None):
        b = np.asarray(b, f32).reshape(A)
        if s is not None:
            b = b * s
        return b.reshape(KC, 128).T  # [128, KC]

    bp = np.concatenate(
        [bT(bh1), bT(bh2, sgn), bT(bf1), bT(bf2, sgn), sgnT],
        axis=1).astype(f32)  # [128, 40]

    xh = np.ascontiguousarray(np.asarray(h_out, f32).T).astype(bf)
    xf = np.ascontiguousarray(np.asarray(fake_region, f32).T).astype(bf)
    # cfes: [B, G, A] -> [A, B, G] *sign -> per-core [A, 2, G, 128]
    cfe = np.asarray(conv_feat_embed, f32).transpose(2, 0, 1) * sgn[:, None, None]
    cfe = cfe.reshape(A, N_CORES, 2, 128, G).transpose(0, 1, 2, 4, 3)
    cfes = np.ascontiguousarray(cfe).astype(f8)  # [A, cores, 2, G, 128]
    cf = np.asarray(conv_feat, f32).astype(f8)

    shared = {
        "wh1t": np.ascontiguousarray(np.asarray(Wh1, f32).T).astype(bf),
        "wh2t": np.ascontiguousarray(np.asarray(Wh2, f32).T).astype(bf),
        "wf1t": np.ascontiguousarray(np.asarray(Wf1, f32).T).astype(bf),
        "wf2t": np.ascontiguousarray(np.asarray(Wf2, f32).T).astype(bf),
        "w2ht": np.ascontiguousarray(np.asarray(W2h, f32).T).astype(bf),
        "bp": bp,
        "awa": awa8.astype(f8),
        "b2h": np.asarray(b2h, f32).reshape(1, R).astype(bf),
    }
    in_maps = []
    for i in range(N_CORES):
        s = slice(i * BS, (i + 1) * BS)
        m = dict(shared)
        m["xh"] = np.ascontiguousarray(xh[:, s])
        m["xf"] = np.ascontiguousarray(xf[:, s])
        m["cfes"] = np.ascontiguousarray(cfes[:, i])
        m["cf"] = np.ascontiguousarray(cf[s])
        in_maps.append(m)
    return in_maps


def kernel(**inputs):
    nc = _get_nc()
    in_maps = _prep_inputs(**inputs)
    res = run_bass_kernel_spmd(nc, in_maps, core_ids=list(range(N_CORES)))
    return np.concatenate([res.results[i]["out"] for i in range(N_CORES)],
                          axis=0)


def run_traced(**inputs):
    nc = _get_nc()
    in_maps = _prep_inputs(**inputs)
    res = run_bass_kernel_spmd(nc, in_maps, core_ids=list(range(N_CORES)),
                               trace=True)
    out = np.concatenate([res.results[i]["out"] for i in range(N_CORES)],
                         axis=0)
    return out, res

